# revision 6
# baseline (speedup 1.0000x reference)
"""Trainium2 Bass kernel for the quaternion-KDE (de la Vallee Poussin) problem.

Math: out[m] = (KAPPA+1) * mean_n( clip(|qy_m . qx_n|, 0, 1-1e-7)^(2*KAPPA) )
with qy/qx unit quaternions from MRP vectors Y [65536,3], X [4096,3], KAPPA=50.

Identity used: cos(2*arccos(c)/2) == c, so the arccos/cos pair in the
reference cancels and the kernel value is (KAPPA+1) * |dot|^100.

Device strategy (8 NeuronCores, Y sharded 8192 queries/core, X replicated):
  - Host precomputes outer-product features Q2[i] = vec(q qT) in R^16 so that
    dot^2 = <QY2_m, QX2_n>  (a K=16 contraction; removes abs+square on device).
  - Features are split hi/lo into bf16 pairs and stacked to K=48
    (hi*hi + hi*lo + lo*hi) so the PE runs at bf16 rate (1 cyc/row) with
    ~1e-5 absolute precision on s = dot^2.
  - Per 128-query block: 8 matmuls [48,128]x[48,512] -> PSUM s-tile,
    ACT Ln (bias 1e-5 guards rounding-negative s), ACT Exp(scale=50,
    accum_out) which also row-sums s^50 over the 4096 samples for free.
  - DVE scales the per-block [128,1] sums by 51/4096 into an output buffer,
    DMA'd out once at the end.
"""

import numpy as np
import ml_dtypes

KAPPA = 50.0
N_X = 4096
N_Y = 65536
N_CORES = 8
M_PER_CORE = N_Y // N_CORES  # 8192
N_MB = M_PER_CORE // 128     # 64 query blocks per core
MM_N = 512                   # matmul moving free dim (one PSUM bank of fp32)
LN_BIAS = 1e-5               # guard: s can round slightly negative

_BUILD_CACHE = {}


def _quat(r):
    r = r.astype(np.float64)
    rr = np.sum(r * r, axis=-1, keepdims=True)
    w = (1.0 - rr) / (1.0 + rr)
    v = 2.0 * r / (1.0 + rr)
    return np.concatenate([w, v], axis=-1)  # [n, 4]


def _features(r):
    q = _quat(r)  # [n,4] float64
    return (q[:, :, None] * q[:, None, :]).reshape(q.shape[0], 16)


def _hilo(a64):
    a32 = a64.astype(np.float32)
    hi = a32.astype(ml_dtypes.bfloat16)
    lo = (a32 - hi.astype(np.float32)).astype(ml_dtypes.bfloat16)
    return hi, lo


def _build(n_mb, n_free):
    """Build the Bass module (SPMD; same program for every core)."""
    key = (n_mb, n_free)
    if key in _BUILD_CACHE:
        return _BUILD_CACHE[key]
    import concourse.tile as tile
    import concourse.mybir as mybir
    from concourse import bacc

    f32 = mybir.dt.float32
    bf16 = mybir.dt.bfloat16
    AF = mybir.ActivationFunctionType

    nc = bacc.Bacc("TRN2", debug=False, target_bir_lowering=False)
    yT = nc.dram_tensor("yt", [48, n_mb * 128], bf16, kind="ExternalInput")
    xT = nc.dram_tensor("xt", [48, n_free], bf16, kind="ExternalInput")
    out = nc.dram_tensor("o", [128, n_mb], f32, kind="ExternalOutput")

    n_half = n_free // 2
    scale_out = float((KAPPA + 1.0) / n_free)

    with tile.TileContext(nc) as tc:
        with (
            tc.tile_pool(name="single", bufs=1) as single,
            tc.tile_pool(name="psum", bufs=2, space="PSUM") as pp,
            tc.tile_pool(name="upool", bufs=2) as up,
            tc.tile_pool(name="epool", bufs=2) as ep,
            tc.tile_pool(name="accp", bufs=4) as accp,
        ):
            y_sb = single.tile([48, n_mb * 128], bf16)
            x_sb = single.tile([48, n_free], bf16)
            ob = single.tile([128, n_mb], f32)
            ln_bias = single.tile([128, 1], f32)
            nc.vector.memset(ln_bias[:], LN_BIAS)
            nc.sync.dma_start(out=y_sb[:], in_=yT[:])
            nc.sync.dma_start(out=x_sb[:], in_=xT[:])

            for mb in range(n_mb):
                u = up.tile([128, n_free], f32)
                for h in range(2):
                    s = pp.tile([128, n_half], f32)
                    for j in range(n_half // MM_N):
                        c = h * (n_half // MM_N) + j
                        nc.tensor.matmul(
                            s[:, j * MM_N:(j + 1) * MM_N],
                            y_sb[:, mb * 128:(mb + 1) * 128],
                            x_sb[:, c * MM_N:(c + 1) * MM_N],
                            start=True,
                            stop=True,
                        )
                    nc.scalar.activation(
                        u[:, h * n_half:(h + 1) * n_half], s[:], AF.Ln,
                        bias=ln_bias[:],
                    )
                e = ep.tile([128, n_free], bf16)
                acc = accp.tile([128, 1], f32)
                nc.scalar.activation(
                    e[:], u[:], AF.Exp, scale=KAPPA, accum_out=acc[:]
                )
                nc.vector.tensor_scalar_mul(ob[:, mb:mb + 1], acc[:], scale_out)

            nc.sync.dma_start(out=out[:], in_=ob[:])

    nc.compile()
    _BUILD_CACHE[key] = nc
    return nc


def _prep_inputs(X, Y):
    """Host-side O(M+N) feature prep -> per-core input maps."""
    fx = _features(np.asarray(X))          # [4096, 16]
    fy = _features(np.asarray(Y))          # [65536, 16]
    xhi, xlo = _hilo(fx)
    yhi, ylo = _hilo(fy)
    # rhs rows pair with lhsT rows: (hiY,hiX), (hiY,loX), (loY,hiX)
    xT = np.concatenate([xhi.T, xlo.T, xhi.T], axis=0)  # [48, 4096]
    in_maps = []
    for c in range(N_CORES):
        sl = slice(c * M_PER_CORE, (c + 1) * M_PER_CORE)
        yT = np.concatenate([yhi[sl].T, yhi[sl].T, ylo[sl].T], axis=0)  # [48, 8192]
        in_maps.append({
            "yt": np.ascontiguousarray(yT),
            "xt": np.ascontiguousarray(xT),
        })
    return in_maps


def kernel(X, Y, trace=False):
    from concourse.bass_utils import run_bass_kernel_spmd

    in_maps = _prep_inputs(X, Y)
    nc = _build(N_MB, N_X)
    res = run_bass_kernel_spmd(
        nc, in_maps, core_ids=list(range(N_CORES)), trace=trace
    )
    outs = []
    for r in res.results:
        o = r["o"]  # [128, n_mb]; out[m] with m = mb*128 + p lives at o[p, mb]
        outs.append(np.asarray(o).T.reshape(-1))
    full = np.concatenate(outs, axis=0).astype(np.float32)
    if trace:
        return full, res
    return full
